# revision 21
# baseline (speedup 1.0000x reference)
"""Trainium2 Bass kernel for nn_Encoder (GRU + input attention), v2.

Shapes (hardcoded): B=32, T=128, N=256, H=512; 8 NeuronCores, batch
sharded 4 examples/core (BL=4).

Two structural changes vs the classic lagged-pipeline formulation:

1. Chunked scan: the GRU map is contracting (the state forgets its
   initial condition at ~0.73x/step), so the T=128 sequence is split
   into S=16 chunks of 8 steps run in LOCKSTEP as extra gate columns;
   chunks with enough history warm up from h=0 over the previous
   chunk's last W=16 steps, while chunks 0 and 1 are held at h0 by a
   z-freeze (z-gate input +60 => z=1 => h'=h exactly in f16) until
   their first real step, making them exact. Serial steps: 24 instead
   of 128. Step k, chunk i processes t = 8*i + k - 16 (m-frame pads
   cover t<0).

2. Matmul-ized attention: with q = tanh(sx/2) (half-angle of
   a = tanh(sx)) and c = tanh(hp):
     tanh(sx+hp) = q + (1-q^2) * sum_{m>=1} lam_m (-q)^(m-1) T_m(c)
   (Chebyshev expansion of the Moebius map (a+c)/(1+ac) in c; the
   coefficients are geometric in q = a/(1+sqrt(1-a^2)) <= 0.87, far
   better than the naive series in a*c whose ratio reaches 0.99).
   Truncated at J=12 with least-squares-fitted taper lam_m; each term
   of the u-contraction is a matmul (stationary T_m(c), moving
   G_m = lam_m v (1-q^2) (-q)^(m-1)), so the 16.7M-element e=tanh(...)
   tensor of the direct formulation never materializes. The lam taper
   rides in per-m scaled (-q) tiles so the G-stream (Pool engine,
   during the scan) and the Chebyshev links (DVE even/odd chains,
   epilogue) are plain tensor_tensor ops in fast DVE modes.

Other tricks: sigmoid+tanh live in one activation table set (exp only
at the final softmax => 2 table loads total); fp8e4m3 z/r recurrent
weights, f16 h-gate weights; gate biases and the per-t x-projections
are folded into identity-matmul PSUM seeds; filler matmuls into a
scratch PSUM bank keep the PE clock at max (the cost model drops to
1.2GHz after any idle gap); the score PSUM partition p = S*o + i is a
permutation of t' = 8*i + o that the host prep undoes, which also
absorbs the reference's alpha.reshape(-1, T, N) flat-reindex quirk.

Layouts per core:
  t_hs    [128, 16, 25, 16] f16  [p, 4j+b, slot, i]; slot k+1 = state
                                 after step k; hs(t) at slot t%8+17,
                                 chunk t//8.
  t_addmx [128, 48, 18, 8]  f16  [p, gate g, m, kr]; t_pad=8m+kr=t+16;
                                 g: z 0:16, r 16:32, h-seed 32:48
                                 (g=4*uc+b); m<2 is the warmup pad.
  t_xh    [128, 16, 18, 8]  f16  same t addressing.
  gates   [128, 16, 16] f32 PSUM (4uc+b, i), seeded from the addmx
                                 slice [:, g, ms:ms+16, kr] (ms=k//8).
  score   [128, 2, 256] f32 PSUM two b per bank, one seed matmul.
"""

import os
import sys

import numpy as np

for _p in ("/root/.axon_site", "/root/.axon_site/_ro/trn_rl_repo",
           "/root/.axon_site/_ro/pypackages", "/opt/trn_rl_repo",
           "/opt/pypackages"):
    if os.path.isdir(_p) and _p not in sys.path:
        sys.path.append(_p)

B, T, N, H = 32, 128, 256, 512
NC = 8           # cores
BL = B // NC     # batch per core (4)
S = 16           # scan chunks
CL = T // S      # chunk length (8)
W = 16           # warmup steps
NP = W // CL     # pad frames (2)
NM = NP + S      # m-frames in addmx (18)
NK = W + CL      # serial steps (24)

LAM = np.array([
    0.99985935, 0.99958375, 0.99593208, 0.99216078, 0.96761417,
    0.95277552, 0.86574698, 0.8334939, 0.63463465, 0.59383626,
    0.29231448, 0.26749125])
J = len(LAM)

_CACHE = {}
DEBUG = os.environ.get("NN_ENC_DEBUG", "0") == "1"
SCAN_ONLY = os.environ.get("NN_ENC_SCAN_ONLY", "0") == "1"


def _build():
    import concourse.bass as bass
    import concourse.bacc as bacc
    import concourse.tile as tile
    import concourse.mybir as mybir

    f16 = mybir.dt.float16
    f32 = mybir.dt.float32
    f8 = mybir.dt.float8e4
    Alu = mybir.AluOpType
    Act = mybir.ActivationFunctionType

    nc = bacc.Bacc("TRN2", target_bir_lowering=False, debug=False)

    # ---- dram I/O ----
    d_data16 = nc.dram_tensor("data16", [BL, T, N], f16, kind="ExternalInput")
    d_dataout = nc.dram_tensor("dataout", [BL, 128, N], f32, kind="ExternalInput")
    d_h0t = nc.dram_tensor("h0t", [128, 16, S], f16, kind="ExternalInput")
    d_R8 = nc.dram_tensor("R8_l", [128, 2, 2, 8, 128], f8, kind="ExternalInput")
    d_Rh = nc.dram_tensor("Rh_l", [128, 4, 4, 128], f16, kind="ExternalInput")
    d_K = nc.dram_tensor("K_l", [128, 2, 12, 128], f16, kind="ExternalInput")
    d_w1 = nc.dram_tensor("w1_l", [128, 128], f16, kind="ExternalInput")
    d_w2 = nc.dram_tensor("w2_l", [128, 4, 128], f16, kind="ExternalInput")
    d_ident = nc.dram_tensor("ident", [128, 128], f16, kind="ExternalInput")
    d_ones = nc.dram_tensor("ones", [128, 128], f16, kind="ExternalInput")
    d_bzr = nc.dram_tensor("bias_zr", [128, 8], f32, kind="ExternalInput")
    d_bh = nc.dram_tensor("bias_h", [128, 4], f32, kind="ExternalInput")
    d_brech = nc.dram_tensor("brech_rep", [128, 16, NM, CL], f16,
                             kind="ExternalInput")
    d_bu = nc.dram_tensor("bias_u", [128, 1], f32, kind="ExternalInput")
    d_v16 = nc.dram_tensor("v16", [128, 1], f32, kind="ExternalInput")
    d_out = nc.dram_tensor("out", [BL, 128, N], f32, kind="ExternalOutput")
    if DEBUG:
        d_hs = nc.dram_tensor("hs_dump", [128, 16, NK + 1, S], f16,
                              kind="ExternalOutput")
        d_sxd = nc.dram_tensor("sx_dump", [128, BL, N], f16,
                               kind="ExternalOutput")
        d_qd = nc.dram_tensor("q_dump", [128, BL, N], f16,
                              kind="ExternalOutput")
        d_cd = nc.dram_tensor("c_dump", [128, BL, CL, S], f16,
                              kind="ExternalOutput")
        d_alp = nc.dram_tensor("alpha_dump", [BL, 128, N], f16,
                               kind="ExternalOutput")

    with tile.TileContext(nc) as tc:
        with (
            tc.tile_pool(name="const", bufs=1) as cpool,
            tc.tile_pool(name="work", bufs=4) as wpool,
            tc.tile_pool(name="mh", bufs=1, space="PSUM") as mhpool,
            tc.tile_pool(name="mhz", bufs=1, space="PSUM") as mhzpool,
            tc.tile_pool(name="mhh", bufs=1, space="PSUM") as mhhpool,
            tc.tile_pool(name="bigps", bufs=2, space="PSUM") as bpool,
            tc.tile_pool(name="scps", bufs=2, space="PSUM") as spool,
            tc.tile_pool(name="fill", bufs=1, space="PSUM") as fpool,
        ):
            # ---- persistent tiles ----
            t_R8 = cpool.tile([128, 2, 2, 8, 128], f8)
            t_Rh = cpool.tile([128, 4, 4, 128], f16)
            t_K = cpool.tile([128, 2, 12, 128], f16)
            t_w1 = cpool.tile([128, 128], f16)
            t_w2 = cpool.tile([128, 4, 128], f16)
            t_ident = cpool.tile([128, 128], f16)
            t_ones = cpool.tile([128, 128], f16)
            t_bzr = cpool.tile([128, 8], f32)
            t_bh = cpool.tile([128, 4], f32)
            t_bu = cpool.tile([128, 1], f32)
            t_v = cpool.tile([128, 1], f32)
            t_d16 = [cpool.tile([128, N], f16, tag=f"d16_{b}", name=f"d16_{b}")
                     for b in range(BL)]
            t_dT = cpool.tile([128, 2, BL, 128], f16)   # dataT [p, nc2, b, t]
            t_addmx = cpool.tile([128, 48, NM, CL], f16)
            t_xh = cpool.tile([128, 16, NM, CL], f16)
            t_sx = cpool.tile([128, BL, N], f16)        # score_x' per b
            t_hs = cpool.tile([128, 16, NK + 1, S], f16)
            # attention setup
            t_q = cpool.tile([128, BL, N], f16)
            t_nq = cpool.tile([128, BL, N], f16)
            t_nqm = [cpool.tile([128, BL, N], f16, tag=f"nqm_{m}",
                                name=f"nqm_{m}") for m in range(2, J + 1)]
            t_q2 = cpool.tile([128, BL, N], f16)
            t_pf = cpool.tile([128, BL, N], f16)
            t_vq = cpool.tile([128, BL, N], f16)
            t_G = [cpool.tile([128, BL, N], f16, tag=f"G_{m}", name=f"G_{m}")
                   for m in range(1, J + 1)]
            t_c1 = cpool.tile([128, BL, CL, S], f16)
            t_ssum = cpool.tile([128, BL], f32)
            t_rinv = cpool.tile([128, BL], f32)

            # warm up the PE clock while DMAs land (fillers have no deps)
            t_fill0 = cpool.tile([128, 64], f16, name="fill_sb")
            nc.vector.memset(t_fill0[:, :], 1.0)

            # ---- DMA in (prologue deps first, epilogue-only last) ----
            nc.sync.dma_start(out=t_ident[:, :], in_=d_ident.ap()[:, :])
            for b in range(BL):
                nc.sync.dma_start(out=t_d16[b][:, :], in_=d_data16.ap()[b, :, :])
            nc.sync.dma_start(out=t_K[:, :, 0:4, :], in_=d_K.ap()[:, :, 0:4, :])
            nc.sync.dma_start(out=t_K[:, :, 4:8, :], in_=d_K.ap()[:, :, 4:8, :])
            nc.sync.dma_start(out=t_K[:, :, 8:12, :],
                              in_=d_K.ap()[:, :, 8:12, :])
            nc.sync.dma_start(out=t_w1[:, :], in_=d_w1.ap()[:, :])
            nc.sync.dma_start(out=t_bzr[:, :], in_=d_bzr.ap()[:, :])
            nc.sync.dma_start(out=t_bh[:, :], in_=d_bh.ap()[:, :])
            nc.sync.dma_start(out=t_bu[:, :], in_=d_bu.ap()[:, :])
            nc.sync.dma_start(out=t_R8[:, :, :, :, :], in_=d_R8.ap()[:, :, :, :, :])
            nc.sync.dma_start(out=t_Rh[:, :, :, :], in_=d_Rh.ap()[:, :, :, :])
            nc.sync.dma_start(out=t_addmx[:, 32:48, :, :],
                              in_=d_brech.ap()[:, :, :, :])
            nc.sync.dma_start(out=t_hs[:, :, 0, :], in_=d_h0t.ap()[:, :, :])
            nc.sync.dma_start(out=t_v[:, :], in_=d_v16.ap()[:, :])
            nc.sync.dma_start(out=t_w2[:, :, :], in_=d_w2.ap()[:, :, :])
            nc.sync.dma_start(out=t_ones[:, :], in_=d_ones.ap()[:, :])

            # warmup pads: z-freeze (sigmoid(60)=1 -> h'=h), r=0, xh=0
            nc.vector.memset(t_addmx[:, 0:16, 0:NP, :], 60.0)
            nc.vector.memset(t_addmx[:, 16:32, 0:NP, :], 0.0)
            nc.vector.memset(t_xh[:, :, 0:NP, :], 0.0)

            # ---- PE p-state fillers: harmless matmuls into a scratch
            # bank keep the tensor engine from dropping out of max clock
            # during dependency gaps (max clock needs 3us continuous) ----
            t_fill = fpool.tile([128, 64], f32, name="fill_ps")

            def emit_fill(n):
                for _ in range(n):
                    nc.tensor.matmul(t_fill[0:64, :], t_fill0[:, :],
                                     t_fill0[:, :], start=True, stop=False,
                                     skip_group_check=True)

            emit_fill(70)

            # ---- prologue: data^T  [p, nc2, b, t] ----
            for b in range(BL):
                for n2 in range(2):
                    ps = bpool.tile([128, 128], f16, tag="bigps")
                    nc.tensor.transpose(ps[:, :],
                                        t_d16[b][:, 128 * n2:128 * (n2 + 1)],
                                        t_ident[:, :])
                    nc.vector.tensor_copy(t_dT[:, n2, b, :], ps[:, :])

            # ---- prologue: mx = data @ K (+biases) -> addmx/xh ----
            for uc in range(12):
                ps = bpool.tile([128, BL, 128], f32, tag="bigps")
                for n2 in range(2):
                    nc.tensor.matmul(ps[:, :, :], t_K[:, n2, uc, :],
                                     t_dT[:, n2, :, :],
                                     start=(n2 == 0), stop=(n2 == 1))
                g, j = divmod(uc, 4)
                if g < 2:
                    dst, bias = t_addmx[:, 4 * uc:4 * uc + 4, NP:NM, :], \
                        t_bzr[:, uc:uc + 1]
                else:
                    dst, bias = t_xh[:, 4 * j:4 * j + 4, NP:NM, :], \
                        t_bh[:, j:j + 1]
                if uc % 2 == 0:
                    nc.scalar.activation(dst, ps[:, :, :], Act.Identity,
                                         bias=bias)
                else:
                    nc.vector.tensor_scalar(dst, ps[:, :, :], bias, None,
                                            Alu.add)

            # ---- prologue: score_x' ----
            for b in range(BL):
                ps = bpool.tile([128, N], f32, tag="bigps")
                nc.tensor.matmul(ps[:, :], t_w1[:, :], t_d16[b][:, :],
                                 start=True, stop=True)
                nc.scalar.activation(t_sx[:, b, :], ps[:, :],
                                     Act.Identity, bias=t_bu[:, :])

            # ---- attention setup ops (independent of the scan; emitted
            # interleaved into scan steps below to fill DVE idle time) ----
            setup_ops = []
            if not SCAN_ONLY:
                setup_ops = [
                    # q = tanh(sx/2) (half-angle of a=tanh(sx)); pref = 1-q^2
                    lambda: nc.scalar.activation(t_q[:, :, :], t_sx[:, :, :],
                                                 Act.Tanh, scale=0.5),
                    lambda: nc.gpsimd.tensor_tensor(
                        t_q2[:, :, :], t_q[:, :, :], t_q[:, :, :], Alu.mult),
                    lambda: nc.vector.tensor_scalar(
                        t_pf[:, :, :], t_q2[:, :, :], -1.0, 1.0,
                        Alu.mult, Alu.add),
                    lambda: nc.vector.tensor_scalar(
                        t_nq[:, :, :], t_q[:, :, :], -1.0, None, Alu.mult),
                    lambda: nc.vector.tensor_scalar(
                        t_vq[:, :, :], t_q[:, :, :], t_v[:, 0:1], None,
                        Alu.mult),
                    lambda: nc.vector.tensor_scalar(
                        t_G[0][:, :, :], t_pf[:, :, :], t_v[:, 0:1],
                        float(LAM[0]), Alu.mult, Alu.mult),
                ]
                # per-m scaled -q tiles carry the lambda taper so both the
                # Pool G-stream and the epilogue Chebyshev links are plain TT
                for m in range(2, J + 1):
                    def _s(m=m):
                        nc.vector.tensor_scalar(
                            t_nqm[m - 2][:, :, :], t_nq[:, :, :],
                            float(LAM[m - 1] / LAM[m - 2]), None, Alu.mult)
                    setup_ops.append(_s)
                for m in range(2, J + 1):
                    def _g(m=m):
                        nc.gpsimd.tensor_tensor(
                            t_G[m - 1][:, :, :], t_G[m - 2][:, :, :],
                            t_nqm[m - 2][:, :, :], Alu.mult)
                    setup_ops.append(_g)
            setup_i = [0]

            def emit_setup():
                if setup_i[0] < len(setup_ops):
                    setup_ops[setup_i[0]]()
                    setup_i[0] += 1

            # ---- main scan loop (NK serial steps) ----
            xhc_t = {}

            def emit_xh_copy(k):
                ms, kr = divmod(k, CL)
                xc = wpool.tile([128, 16, S], f16, tag="xhc", bufs=4)
                nc.vector.tensor_copy(xc[:, :, :], t_xh[:, :, ms:ms + S, kr])
                xhc_t[k] = xc

            emit_xh_copy(0)
            emit_xh_copy(1)
            hp = bpool.tile([128, BL, CL, S], f32, tag="bigps", name="hp_ps")

            def emit_hp(o):
                # hp[u, b, o, i] = sum_h hs[t=8i+o, b, h] w2[h, u]
                for hc in range(4):
                    nc.tensor.matmul(hp[:, :, o, :], t_w2[:, hc, :],
                                     t_hs[:, 4 * hc:4 * hc + 4, o + 17, :],
                                     start=(hc == 0), stop=(hc == 3),
                                     skip_group_check=True)

            for k in range(NK):
                ms, kr = divmod(k, CL)

                mhr = mhpool.tile([128, 16, S], f32)
                mhz = mhzpool.tile([128, 16, S], f32)
                mhh = mhhpool.tile([128, 16, S], f32)
                # r bank first (chain-critical), then h, then z
                nc.tensor.matmul(mhr[:, :, :], t_ident[:, :],
                                 t_addmx[:, 16:32, ms:ms + S, kr],
                                 start=True, stop=False, skip_group_check=True)
                for kc in range(4):
                    for uc in range(4):
                        nc.tensor.matmul(mhr[:, 4 * uc:4 * uc + 4, :],
                                         t_R8[:, kc // 2, kc % 2, uc + 4, :],
                                         t_hs[:, 4 * kc:4 * kc + 4, k, :],
                                         start=False,
                                         stop=(uc == 3 and kc == 3),
                                         skip_group_check=True)
                nc.tensor.matmul(mhh[:, :, :], t_ident[:, :],
                                 t_addmx[:, 32:48, ms:ms + S, kr],
                                 start=True, stop=False, skip_group_check=True)
                for uc in range(4):
                    for kc in range(4):
                        nc.tensor.matmul(mhh[:, 4 * uc:4 * uc + 4, :],
                                         t_Rh[:, kc, uc, :],
                                         t_hs[:, 4 * kc:4 * kc + 4, k, :],
                                         start=False,
                                         stop=(uc == 3 and kc == 3),
                                         skip_group_check=True)
                nc.tensor.matmul(mhz[:, :, :], t_ident[:, :],
                                 t_addmx[:, 0:16, ms:ms + S, kr],
                                 start=True, stop=False, skip_group_check=True)
                for uc in range(4):
                    for kc in range(4):
                        nc.tensor.matmul(mhz[:, 4 * uc:4 * uc + 4, :],
                                         t_R8[:, kc // 2, kc % 2, uc, :],
                                         t_hs[:, 4 * kc:4 * kc + 4, k, :],
                                         start=False,
                                         stop=(uc == 3 and kc == 3),
                                         skip_group_check=True)

                emit_fill(12)
                xhc = xhc_t.pop(k)

                gate_prio = tc.high_priority(offset=200000)
                gate_prio.__enter__()

                t2 = wpool.tile([128, 16, S], f16, tag="t2")
                t3 = wpool.tile([128, 16, S], f16, tag="t3")
                cg = wpool.tile([128, 16, S], f16, tag="cg")
                tz = wpool.tile([128, 16, S], f16, tag="tz")
                uh = wpool.tile([128, 16, S], f16, tag="uh")
                st = wpool.tile([128, 16, S], f16, tag="st")
                sc = wpool.tile([128, 16, S], f16, tag="sc")

                tr = wpool.tile([128, 16, S], f16, tag="tr")
                nc.scalar.activation(tr[:, :, :], mhr[:, :, :], Act.Sigmoid)
                nc.vector.tensor_tensor(t2[:, :, :], tr[:, :, :],
                                        mhh[:, :, :], Alu.mult)
                nc.vector.tensor_tensor(t3[:, :, :], t2[:, :, :],
                                        xhc[:, :, :], Alu.add)
                nc.scalar.activation(tz[:, :, :], mhz[:, :, :], Act.Sigmoid)
                nc.vector.tensor_tensor(uh[:, :, :], tz[:, :, :],
                                        t_hs[:, :, k, :], Alu.mult)
                nc.vector.tensor_scalar(st[:, :, :], tz[:, :, :], -1.0, 1.0,
                                        Alu.mult, Alu.add)
                if k + 2 < NK:
                    emit_xh_copy(k + 2)
                nc.scalar.activation(cg[:, :, :], t3[:, :, :], Act.Tanh)
                nc.vector.tensor_tensor(sc[:, :, :], st[:, :, :],
                                        cg[:, :, :], Alu.mult)
                nc.vector.tensor_tensor(t_hs[:, 0:8, k + 1, :],
                                        uh[:, 0:8, :], sc[:, 0:8, :], Alu.add)
                nc.vector.tensor_tensor(t_hs[:, 8:16, k + 1, :],
                                        uh[:, 8:16, :], sc[:, 8:16, :],
                                        Alu.add)
                gate_prio.__exit__(None, None, None)

                if k >= 4:
                    emit_setup()
                if not SCAN_ONLY and NK - CL <= k < NK:
                    emit_hp(k - (NK - CL))

            while setup_i[0] < len(setup_ops):
                emit_setup()

            # ---- attention epilogue ----
            if not SCAN_ONLY:
                HO = CL // 2
                HA, HB = slice(0, HO), slice(HO, CL)
                for sl in (HA, HB):
                    nc.scalar.activation(t_c1[:, :, sl, :], hp[:, :, sl, :],
                                         Act.Tanh)

                emit_fill(40)
                ps_pair = [spool.tile([128, 2, N], f32, tag="scps",
                                      name=f"scps_{p}") for p in range(2)]
                for p in range(2):
                    nc.tensor.matmul(ps_pair[p][:, :, :], t_ones[:, :],
                                     t_vq[:, 2 * p:2 * p + 2, :],
                                     start=True, stop=False,
                                     skip_group_check=True)
                # Raw Chebyshev even/odd chains (plain TT links, 2x mode):
                # T_m = (2 T_2) T_{m-2} - T_{m-4}; lambda lives in the G tiles
                tmt = {1: t_c1}
                c2t = wpool.tile([128, BL, CL, S], f16, tag="tchw", bufs=6)
                t2c = wpool.tile([128, BL, CL, S], f16, tag="tcheb", bufs=12)
                t2d = wpool.tile([128, BL, CL, S], f16, tag="tchw", bufs=6)
                tmt[2] = t2c
                for sl in (HA, HB):
                    nc.vector.tensor_tensor(c2t[:, :, sl, :],
                                            t_c1[:, :, sl, :],
                                            t_c1[:, :, sl, :], Alu.mult)
                    nc.vector.tensor_scalar(t2c[:, :, sl, :],
                                            c2t[:, :, sl, :],
                                            2.0, -1.0, Alu.mult, Alu.add)
                    nc.vector.tensor_scalar(t2d[:, :, sl, :],
                                            t2c[:, :, sl, :],
                                            2.0, None, Alu.mult)

                def emit_cheb(m, sl):  # T_m = t2d * T_{m-2} - T_{m-4}
                    if m not in tmt:
                        tmt[m] = wpool.tile([128, BL, CL, S], f16,
                                            tag="tcheb", bufs=12,
                                            name=f"tm_{m}")
                    tw = wpool.tile([128, BL, CL, S], f16, tag="tchw", bufs=6)
                    nc.vector.tensor_tensor(
                        tw[:, :, sl, :], t2d[:, :, sl, :],
                        tmt[m - 2][:, :, sl, :], Alu.mult)
                    if m == 4:  # T_0 = 1
                        nc.vector.tensor_scalar(
                            tmt[m][:, :, sl, :], tw[:, :, sl, :], 1.0, -1.0,
                            Alu.mult, Alu.add)
                    else:  # m=3 uses T_{-1} = T_1
                        nc.vector.tensor_tensor(
                            tmt[m][:, :, sl, :], tw[:, :, sl, :],
                            tmt[m - 4 if m > 4 else 1][:, :, sl, :],
                            Alu.subtract)

                for sl in (HA, HB):
                    for m in range(3, J + 1):
                        emit_cheb(m, sl)
                for m in range(1, J + 1):
                    for b in range(BL):
                        nc.tensor.matmul(ps_pair[b // 2][:, b % 2, :],
                                         tmt[m][:, b, :, :], t_G[m - 1][:, b, :],
                                         start=False, stop=(m == J),
                                         skip_group_check=True)
                    emit_fill(4)

                # softmax over n (free dim) + final data multiply
                for b in range(BL):
                    ex = wpool.tile([128, N], f16, tag="expv")
                    nc.scalar.activation(ex[:, :], ps_pair[b // 2][:, b % 2, :],
                                         Act.Exp,
                                         accum_out=t_ssum[:, b:b + 1])
                    nc.vector.reciprocal(t_rinv[:, b:b + 1],
                                         t_ssum[:, b:b + 1])
                    alpha = wpool.tile([128, N], f16, tag="alphav")
                    nc.vector.tensor_scalar(
                        alpha[:, :], ex[:, :], t_rinv[:, b:b + 1], None,
                        Alu.mult)
                    dt_ = wpool.tile([128, N], f32, tag="dmul")
                    ot = wpool.tile([128, N], f32, tag="omul")
                    nc.sync.dma_start(out=dt_[:, :], in_=d_dataout.ap()[b, :, :])
                    eng = nc.vector if b % 2 == 0 else nc.gpsimd
                    eng.tensor_tensor(ot[:, :], dt_[:, :], alpha[:, :],
                                      Alu.mult)
                    nc.sync.dma_start(out=d_out.ap()[b, :, :], in_=ot[:, :])
                    if DEBUG:
                        nc.sync.dma_start(out=d_alp.ap()[b, :, :],
                                          in_=alpha[:, :])
            else:
                for b in range(BL):
                    dt_ = wpool.tile([128, N], f32, tag="dmul")
                    ot = wpool.tile([128, N], f32, tag="omul")
                    nc.sync.dma_start(out=dt_[:, :], in_=d_dataout.ap()[b, :, :])
                    nc.vector.tensor_tensor(ot[:, :], dt_[:, :], dt_[:, :],
                                            Alu.mult)
                    nc.sync.dma_start(out=d_out.ap()[b, :, :], in_=ot[:, :])

            if DEBUG:
                nc.sync.dma_start(out=d_hs.ap()[:, :, :, :],
                                  in_=t_hs[:, :, :, :])
                nc.sync.dma_start(out=d_sxd.ap()[:, :, :], in_=t_sx[:, :, :])
                if not SCAN_ONLY:
                    nc.sync.dma_start(out=d_qd.ap()[:, :, :], in_=t_q[:, :, :])
                    nc.sync.dma_start(out=d_cd.ap()[:, :, :, :],
                                      in_=t_c1[:, :, :, :])

    nc.compile()
    return nc


def _prep_inputs(data, h0, gru_kernel, gru_rkernel, gru_bias,
                 w1_w, w1_b, w2_w, w2_b, v_w, v_b):
    f16 = np.float16
    f32 = np.float32

    import ml_dtypes
    R_all = np.ascontiguousarray(
        gru_rkernel.reshape(4, 128, 12, 128).transpose(1, 0, 2, 3))
    R8_l = np.ascontiguousarray(
        R_all[:, :, 0:8, :].astype(ml_dtypes.float8_e4m3).reshape(
            128, 2, 2, 8, 128))
    Rh_l = R_all[:, :, 8:12, :].astype(f16)
    K_l = np.ascontiguousarray(
        gru_kernel.astype(f16).reshape(2, 128, 12, 128).transpose(1, 0, 2, 3))
    w1_l = w1_w.astype(f16)
    w2_l = np.ascontiguousarray(
        w2_w.astype(f16).reshape(4, 128, 128).transpose(1, 0, 2))
    ident = np.eye(128, dtype=f16)
    ones = np.ones((128, 128), f16)

    b_in, b_rec = gru_bias[0].astype(f32), gru_bias[1].astype(f32)
    bzr = (b_in + b_rec)[:1024].reshape(8, 128).T.copy()      # [128, 8]
    bh = b_in[1024:].reshape(4, 128).T.copy()                 # [128, 4]
    brech16 = np.zeros((128, 16), f16)
    for j in range(4):
        for bb in range(4):
            brech16[:, 4 * j + bb] = b_rec[1024 + 128 * j:1024 + 128 * (j + 1)]
    brech_rep = np.ascontiguousarray(np.broadcast_to(
        brech16[:, :, None, None], (128, 16, W // (T // S) + S, T // S))
        ).astype(f16)
    bu = (w1_b + w2_b).astype(f32).reshape(128, 1)
    v16 = v_w[:, 0].astype(f32).reshape(128, 1)

    data16 = data.astype(f16)

    # output row perm: psum partition p = S*o + i  <->  t' = CL*i + o
    pp = np.arange(128)
    tprime = (T // S) * (pp % S) + pp // S

    per_core = []
    for c in range(NC):
        sl = slice(BL * c, BL * (c + 1))
        h0t = np.zeros((128, 16, S), f16)
        for j in range(4):
            for bb in range(BL):
                for ch in range(W // (T // S)):
                    h0t[:, 4 * j + bb, ch] = \
                        h0[BL * c + bb, 128 * j:128 * (j + 1)]
        # dataout[l, p, :] = data[bg, tg] with flat = t'*32 + 4c + l
        dataout = np.empty((BL, 128, N), f32)
        for l in range(BL):
            flat = tprime * 32 + 4 * c + l
            dataout[l] = data[flat // 128, flat % 128, :]
        per_core.append({
            "data16": data16[sl], "dataout": dataout, "h0t": h0t,
            "R8_l": R8_l, "Rh_l": Rh_l, "K_l": K_l, "w1_l": w1_l,
            "w2_l": w2_l, "ident": ident, "ones": ones, "bias_zr": bzr,
            "bias_h": bh, "brech_rep": brech_rep, "bias_u": bu, "v16": v16,
        })
    return per_core


def kernel(**inputs):
    from concourse.bass_utils import run_bass_kernel_spmd

    if "nc" not in _CACHE:
        _CACHE["nc"] = _build()
    nc = _CACHE["nc"]

    args = {k: np.asarray(v) for k, v in inputs.items()}
    per_core = _prep_inputs(
        args["data"], args["h0"], args["gru_kernel"], args["gru_rkernel"],
        args["gru_bias"], args["w1_w"], args["w1_b"], args["w2_w"],
        args["w2_b"], args["v_w"], args["v_b"])

    if "warm" not in _CACHE:
        run_bass_kernel_spmd(nc, per_core, core_ids=list(range(NC)))
        _CACHE["warm"] = True
    res = run_bass_kernel_spmd(nc, per_core, core_ids=list(range(NC)))
    _CACHE["last_res"] = res

    pp = np.arange(128)
    tprime = (T // S) * (pp % S) + pp // S
    out = np.empty((B, T, N), np.float32)
    for c in range(NC):
        o = res.results[c]["out"]
        for l in range(BL):
            flat = tprime * 32 + 4 * c + l
            out[flat // 128, flat % 128, :] = o[l]
    return out


# revision 22
# speedup vs baseline: 1.0442x; 1.0442x over previous
"""Trainium2 Bass kernel for nn_Encoder (GRU + input attention), v2.

Shapes (hardcoded): B=32, T=128, N=256, H=512; 8 NeuronCores, batch
sharded 4 examples/core (BL=4).

Two structural changes vs the classic lagged-pipeline formulation:

1. Chunked scan: the GRU map is contracting (the state forgets its
   initial condition at ~0.73x/step), so the T=128 sequence is split
   into S=16 chunks of 8 steps run in LOCKSTEP as extra gate columns;
   chunks with enough history warm up from h=0 over the previous
   chunk's last W=16 steps, while chunks 0 and 1 are held at h0 by a
   z-freeze (z-gate input +60 => z=1 => h'=h exactly in f16) until
   their first real step, making them exact. Serial steps: 24 instead
   of 128. Step k, chunk i processes t = 8*i + k - 16 (m-frame pads
   cover t<0).

2. Matmul-ized attention: with q = tanh(sx/2) (half-angle of
   a = tanh(sx)) and c = tanh(hp):
     tanh(sx+hp) = q + (1-q^2) * sum_{m>=1} lam_m (-q)^(m-1) T_m(c)
   (Chebyshev expansion of the Moebius map (a+c)/(1+ac) in c; the
   coefficients are geometric in q = a/(1+sqrt(1-a^2)) <= 0.87, far
   better than the naive series in a*c whose ratio reaches 0.99).
   Truncated at J=12 with least-squares-fitted taper lam_m; each term
   of the u-contraction is a matmul (stationary T_m(c), moving
   G_m = lam_m v (1-q^2) (-q)^(m-1)), so the 16.7M-element e=tanh(...)
   tensor of the direct formulation never materializes. The lam taper
   rides in per-m scaled (-q) tiles so the G-stream (Pool engine,
   during the scan) and the Chebyshev links (DVE even/odd chains,
   epilogue) are plain tensor_tensor ops in fast DVE modes.

Other tricks: sigmoid+tanh live in one activation table set (exp only
at the final softmax => 2 table loads total); fp8e4m3 z/r recurrent
weights, f16 h-gate weights; gate biases and the per-t x-projections
are folded into identity-matmul PSUM seeds; filler matmuls into a
scratch PSUM bank keep the PE clock at max (the cost model drops to
1.2GHz after any idle gap); the score PSUM partition p = S*o + i is a
permutation of t' = 8*i + o that the host prep undoes, which also
absorbs the reference's alpha.reshape(-1, T, N) flat-reindex quirk.

Layouts per core:
  t_hs    [128, 16, 25, 16] f16  [p, 4j+b, slot, i]; slot k+1 = state
                                 after step k; hs(t) at slot t%8+17,
                                 chunk t//8.
  t_addmx [128, 48, 18, 8]  f16  [p, gate g, m, kr]; t_pad=8m+kr=t+16;
                                 g: z 0:16, r 16:32, h-seed 32:48
                                 (g=4*uc+b); m<2 is the warmup pad.
  t_xh    [128, 16, 18, 8]  f16  same t addressing.
  gates   [128, 16, 16] f32 PSUM (4uc+b, i), seeded from the addmx
                                 slice [:, g, ms:ms+16, kr] (ms=k//8).
  score   [128, 2, 256] f32 PSUM two b per bank, one seed matmul.
"""

import os
import sys

import numpy as np

for _p in ("/root/.axon_site", "/root/.axon_site/_ro/trn_rl_repo",
           "/root/.axon_site/_ro/pypackages", "/opt/trn_rl_repo",
           "/opt/pypackages"):
    if os.path.isdir(_p) and _p not in sys.path:
        sys.path.append(_p)

B, T, N, H = 32, 128, 256, 512
NC = 8           # cores
BL = B // NC     # batch per core (4)
S = 16           # scan chunks
CL = T // S      # chunk length (8)
W = 16           # warmup steps
NP = W // CL     # pad frames (2)
NM = NP + S      # m-frames in addmx (18)
NK = W + CL      # serial steps (24)

LAM = np.array([
    0.99985935, 0.99958375, 0.99593208, 0.99216078, 0.96761417,
    0.95277552, 0.86574698, 0.8334939, 0.63463465, 0.59383626,
    0.29231448, 0.26749125])
J = len(LAM)

_CACHE = {}
DEBUG = os.environ.get("NN_ENC_DEBUG", "0") == "1"
SCAN_ONLY = os.environ.get("NN_ENC_SCAN_ONLY", "0") == "1"


def _build():
    import concourse.bass as bass
    import concourse.bacc as bacc
    import concourse.tile as tile
    import concourse.mybir as mybir

    f16 = mybir.dt.float16
    f32 = mybir.dt.float32
    f8 = mybir.dt.float8e4
    Alu = mybir.AluOpType
    Act = mybir.ActivationFunctionType

    nc = bacc.Bacc("TRN2", target_bir_lowering=False, debug=False)

    # ---- dram I/O ----
    d_data16 = nc.dram_tensor("data16", [BL, T, N], f16, kind="ExternalInput")
    d_dataout = nc.dram_tensor("dataout", [BL, 128, N], f32, kind="ExternalInput")
    d_h0t = nc.dram_tensor("h0t", [128, 16, S], f16, kind="ExternalInput")
    d_R8 = nc.dram_tensor("R8_l", [128, 2, 2, 8, 128], f8, kind="ExternalInput")
    d_Rh = nc.dram_tensor("Rh_l", [128, 4, 4, 128], f16, kind="ExternalInput")
    d_K = nc.dram_tensor("K_l", [128, 2, 12, 128], f16, kind="ExternalInput")
    d_w1 = nc.dram_tensor("w1_l", [128, 128], f16, kind="ExternalInput")
    d_w2 = nc.dram_tensor("w2_l", [128, 4, 128], f16, kind="ExternalInput")
    d_ident = nc.dram_tensor("ident", [128, 128], f16, kind="ExternalInput")
    d_ones = nc.dram_tensor("ones", [128, 128], f16, kind="ExternalInput")
    d_bzr = nc.dram_tensor("bias_zr", [128, 8], f32, kind="ExternalInput")
    d_bh = nc.dram_tensor("bias_h", [128, 4], f32, kind="ExternalInput")
    d_brech = nc.dram_tensor("brech_rep", [128, 16, NM, CL], f16,
                             kind="ExternalInput")
    d_bu = nc.dram_tensor("bias_u", [128, 1], f32, kind="ExternalInput")
    d_v16 = nc.dram_tensor("v16", [128, 1], f32, kind="ExternalInput")
    d_out = nc.dram_tensor("out", [BL, 128, N], f32, kind="ExternalOutput")
    if DEBUG:
        d_hs = nc.dram_tensor("hs_dump", [128, 16, NK + 1, S], f16,
                              kind="ExternalOutput")
        d_sxd = nc.dram_tensor("sx_dump", [128, BL, N], f16,
                               kind="ExternalOutput")
        d_qd = nc.dram_tensor("q_dump", [128, BL, N], f16,
                              kind="ExternalOutput")
        d_cd = nc.dram_tensor("c_dump", [128, BL, CL, S], f16,
                              kind="ExternalOutput")
        d_alp = nc.dram_tensor("alpha_dump", [BL, 128, N], f16,
                               kind="ExternalOutput")

    with tile.TileContext(nc) as tc:
        with (
            tc.tile_pool(name="const", bufs=1) as cpool,
            tc.tile_pool(name="work", bufs=4) as wpool,
            tc.tile_pool(name="mh", bufs=1, space="PSUM") as mhpool,
            tc.tile_pool(name="mhz", bufs=1, space="PSUM") as mhzpool,
            tc.tile_pool(name="mhh", bufs=1, space="PSUM") as mhhpool,
            tc.tile_pool(name="bigps", bufs=2, space="PSUM") as bpool,
            tc.tile_pool(name="scps", bufs=2, space="PSUM") as spool,
            tc.tile_pool(name="fill", bufs=1, space="PSUM") as fpool,
        ):
            # ---- persistent tiles ----
            t_R8 = cpool.tile([128, 2, 2, 8, 128], f8)
            t_Rh = cpool.tile([128, 4, 4, 128], f16)
            t_K = cpool.tile([128, 2, 12, 128], f16)
            t_w1 = cpool.tile([128, 128], f16)
            t_w2 = cpool.tile([128, 4, 128], f16)
            t_ident = cpool.tile([128, 128], f16)
            t_ones = cpool.tile([128, 128], f16)
            t_bzr = cpool.tile([128, 8], f32)
            t_bh = cpool.tile([128, 4], f32)
            t_bu = cpool.tile([128, 1], f32)
            t_v = cpool.tile([128, 1], f32)
            t_d16 = [cpool.tile([128, N], f16, tag=f"d16_{b}", name=f"d16_{b}")
                     for b in range(BL)]
            t_dT = cpool.tile([128, 2, BL, 128], f16)   # dataT [p, nc2, b, t]
            t_addmx = cpool.tile([128, 48, NM, CL], f16)
            t_xh = cpool.tile([128, 16, NM, CL], f16)
            t_sx = cpool.tile([128, BL, N], f16)        # score_x' per b
            t_hs = cpool.tile([128, 16, NK + 1, S], f16)
            # attention setup
            t_q = cpool.tile([128, BL, N], f16)
            t_nq = cpool.tile([128, BL, N], f16)
            t_nqm = [cpool.tile([128, BL, N], f16, tag=f"nqm_{m}",
                                name=f"nqm_{m}") for m in range(2, J + 1)]
            t_q2 = cpool.tile([128, BL, N], f16)
            t_pf = cpool.tile([128, BL, N], f16)
            t_vq = cpool.tile([128, BL, N], f16)
            t_G = [cpool.tile([128, BL, N], f16, tag=f"G_{m}", name=f"G_{m}")
                   for m in range(1, J + 1)]
            t_c1 = cpool.tile([128, BL, CL, S], f16)
            t_ssum = cpool.tile([128, BL], f32)
            t_rinv = cpool.tile([128, BL], f32)

            # warm up the PE clock while DMAs land (fillers have no deps)
            t_fill0 = cpool.tile([128, 64], f16, name="fill_sb")
            nc.vector.memset(t_fill0[:, :], 1.0)

            # ---- DMA in (prologue deps first, epilogue-only last) ----
            nc.sync.dma_start(out=t_ident[:, :], in_=d_ident.ap()[:, :])
            for b in range(BL):
                nc.sync.dma_start(out=t_d16[b][:, :], in_=d_data16.ap()[b, :, :])
            nc.sync.dma_start(out=t_K[:, :, 0:4, :], in_=d_K.ap()[:, :, 0:4, :])
            nc.sync.dma_start(out=t_K[:, :, 4:8, :], in_=d_K.ap()[:, :, 4:8, :])
            nc.sync.dma_start(out=t_K[:, :, 8:12, :],
                              in_=d_K.ap()[:, :, 8:12, :])
            nc.sync.dma_start(out=t_w1[:, :], in_=d_w1.ap()[:, :])
            nc.sync.dma_start(out=t_bzr[:, :], in_=d_bzr.ap()[:, :])
            nc.sync.dma_start(out=t_bh[:, :], in_=d_bh.ap()[:, :])
            nc.sync.dma_start(out=t_bu[:, :], in_=d_bu.ap()[:, :])
            nc.sync.dma_start(out=t_R8[:, :, :, :, :], in_=d_R8.ap()[:, :, :, :, :])
            nc.sync.dma_start(out=t_Rh[:, :, :, :], in_=d_Rh.ap()[:, :, :, :])
            nc.sync.dma_start(out=t_addmx[:, 32:48, :, :],
                              in_=d_brech.ap()[:, :, :, :])
            nc.sync.dma_start(out=t_hs[:, :, 0, :], in_=d_h0t.ap()[:, :, :])
            nc.sync.dma_start(out=t_v[:, :], in_=d_v16.ap()[:, :])
            nc.sync.dma_start(out=t_w2[:, :, :], in_=d_w2.ap()[:, :, :])
            nc.sync.dma_start(out=t_ones[:, :], in_=d_ones.ap()[:, :])

            # warmup pads: z-freeze (sigmoid(60)=1 -> h'=h), r=0, xh=0
            nc.vector.memset(t_addmx[:, 0:16, 0:NP, :], 60.0)
            nc.vector.memset(t_addmx[:, 16:32, 0:NP, :], 0.0)
            nc.vector.memset(t_xh[:, :, 0:NP, :], 0.0)

            # ---- PE p-state fillers: harmless matmuls into a scratch
            # bank keep the tensor engine from dropping out of max clock
            # during dependency gaps (max clock needs 3us continuous) ----
            t_fill = fpool.tile([128, 64], f32, name="fill_ps")

            def emit_fill(n):
                for _ in range(n):
                    nc.tensor.matmul(t_fill[0:64, :], t_fill0[:, :],
                                     t_fill0[:, :], start=True, stop=False,
                                     skip_group_check=True)

            emit_fill(70)

            # ---- prologue: data^T  [p, nc2, b, t] ----
            for b in range(BL):
                for n2 in range(2):
                    ps = bpool.tile([128, 128], f16, tag="bigps")
                    nc.tensor.transpose(ps[:, :],
                                        t_d16[b][:, 128 * n2:128 * (n2 + 1)],
                                        t_ident[:, :])
                    nc.vector.tensor_copy(t_dT[:, n2, b, :], ps[:, :])

            # ---- prologue: mx = data @ K (+biases) -> addmx/xh ----
            for uc in range(12):
                ps = bpool.tile([128, BL, 128], f32, tag="bigps")
                for n2 in range(2):
                    nc.tensor.matmul(ps[:, :, :], t_K[:, n2, uc, :],
                                     t_dT[:, n2, :, :],
                                     start=(n2 == 0), stop=(n2 == 1))
                g, j = divmod(uc, 4)
                if g < 2:
                    dst, bias = t_addmx[:, 4 * uc:4 * uc + 4, NP:NM, :], \
                        t_bzr[:, uc:uc + 1]
                else:
                    dst, bias = t_xh[:, 4 * j:4 * j + 4, NP:NM, :], \
                        t_bh[:, j:j + 1]
                if uc % 2 == 0:
                    nc.scalar.activation(dst, ps[:, :, :], Act.Identity,
                                         bias=bias)
                else:
                    nc.vector.tensor_scalar(dst, ps[:, :, :], bias, None,
                                            Alu.add)

            # ---- prologue: score_x' ----
            for b in range(BL):
                ps = bpool.tile([128, N], f32, tag="bigps")
                nc.tensor.matmul(ps[:, :], t_w1[:, :], t_d16[b][:, :],
                                 start=True, stop=True)
                nc.scalar.activation(t_sx[:, b, :], ps[:, :],
                                     Act.Identity, bias=t_bu[:, :])

            # ---- attention setup ops (independent of the scan; emitted
            # interleaved into scan steps below to fill DVE idle time) ----
            setup_ops = []
            if not SCAN_ONLY:
                setup_ops = [
                    # q = tanh(sx/2) (half-angle of a=tanh(sx)); pref = 1-q^2
                    lambda: nc.scalar.activation(t_q[:, :, :], t_sx[:, :, :],
                                                 Act.Tanh, scale=0.5),
                    lambda: nc.gpsimd.tensor_tensor(
                        t_q2[:, :, :], t_q[:, :, :], t_q[:, :, :], Alu.mult),
                    lambda: nc.vector.tensor_scalar(
                        t_pf[:, :, :], t_q2[:, :, :], -1.0, 1.0,
                        Alu.mult, Alu.add),
                    lambda: nc.vector.tensor_scalar(
                        t_nq[:, :, :], t_q[:, :, :], -1.0, None, Alu.mult),
                    lambda: nc.vector.tensor_scalar(
                        t_vq[:, :, :], t_q[:, :, :], t_v[:, 0:1], None,
                        Alu.mult),
                    lambda: nc.vector.tensor_scalar(
                        t_G[0][:, :, :], t_pf[:, :, :], t_v[:, 0:1],
                        float(LAM[0]), Alu.mult, Alu.mult),
                ]
                # per-m scaled -q tiles carry the lambda taper so both the
                # Pool G-stream and the epilogue Chebyshev links are plain TT
                for m in range(2, J + 1):
                    def _s(m=m):
                        nc.vector.tensor_scalar(
                            t_nqm[m - 2][:, :, :], t_nq[:, :, :],
                            float(LAM[m - 1] / LAM[m - 2]), None, Alu.mult)
                    setup_ops.append(_s)
                for m in range(2, J + 1):
                    def _g(m=m):
                        nc.gpsimd.tensor_tensor(
                            t_G[m - 1][:, :, :], t_G[m - 2][:, :, :],
                            t_nqm[m - 2][:, :, :], Alu.mult)
                    setup_ops.append(_g)
            setup_i = [0]

            def emit_setup():
                if setup_i[0] < len(setup_ops):
                    setup_ops[setup_i[0]]()
                    setup_i[0] += 1

            # ---- main scan loop (NK serial steps) ----
            xhc_t = {}

            def emit_xh_copy(k):
                ms, kr = divmod(k, CL)
                xc = wpool.tile([128, 16, S], f16, tag="xhc", bufs=4)
                nc.vector.tensor_copy(xc[:, :, :], t_xh[:, :, ms:ms + S, kr])
                xhc_t[k] = xc

            emit_xh_copy(0)
            emit_xh_copy(1)
            hp = bpool.tile([128, BL, CL, S], f32, tag="bigps", name="hp_ps")

            def emit_hp(o):
                # hp[u, b, o, i] = sum_h hs[t=8i+o, b, h] w2[h, u]
                for hc in range(4):
                    nc.tensor.matmul(hp[:, :, o, :], t_w2[:, hc, :],
                                     t_hs[:, 4 * hc:4 * hc + 4, o + 17, :],
                                     start=(hc == 0), stop=(hc == 3),
                                     skip_group_check=True)

            for k in range(NK):
                ms, kr = divmod(k, CL)

                mhr = mhpool.tile([128, 16, S], f32)
                mhz = mhzpool.tile([128, 16, S], f32)
                mhh = mhhpool.tile([128, 16, S], f32)
                # r bank first (chain-critical), then h, then z
                nc.tensor.matmul(mhr[:, :, :], t_ident[:, :],
                                 t_addmx[:, 16:32, ms:ms + S, kr],
                                 start=True, stop=False, skip_group_check=True)
                for kc in range(4):
                    for uc in range(4):
                        nc.tensor.matmul(mhr[:, 4 * uc:4 * uc + 4, :],
                                         t_R8[:, kc // 2, kc % 2, uc + 4, :],
                                         t_hs[:, 4 * kc:4 * kc + 4, k, :],
                                         start=False,
                                         stop=(uc == 3 and kc == 3),
                                         skip_group_check=True)
                nc.tensor.matmul(mhh[:, :, :], t_ident[:, :],
                                 t_addmx[:, 32:48, ms:ms + S, kr],
                                 start=True, stop=False, skip_group_check=True)
                for uc in range(4):
                    for kc in range(4):
                        nc.tensor.matmul(mhh[:, 4 * uc:4 * uc + 4, :],
                                         t_Rh[:, kc, uc, :],
                                         t_hs[:, 4 * kc:4 * kc + 4, k, :],
                                         start=False,
                                         stop=(uc == 3 and kc == 3),
                                         skip_group_check=True)
                nc.tensor.matmul(mhz[:, :, :], t_ident[:, :],
                                 t_addmx[:, 0:16, ms:ms + S, kr],
                                 start=True, stop=False, skip_group_check=True)
                for uc in range(4):
                    for kc in range(4):
                        nc.tensor.matmul(mhz[:, 4 * uc:4 * uc + 4, :],
                                         t_R8[:, kc // 2, kc % 2, uc, :],
                                         t_hs[:, 4 * kc:4 * kc + 4, k, :],
                                         start=False,
                                         stop=(uc == 3 and kc == 3),
                                         skip_group_check=True)

                emit_fill(12)
                xhc = xhc_t.pop(k)

                gate_prio = tc.high_priority(offset=200000)
                gate_prio.__enter__()

                t2 = wpool.tile([128, 16, S], f16, tag="t2")
                t3 = wpool.tile([128, 16, S], f16, tag="t3")
                cg = wpool.tile([128, 16, S], f16, tag="cg")
                tz = wpool.tile([128, 16, S], f16, tag="tz")
                uh = wpool.tile([128, 16, S], f16, tag="uh")
                st = wpool.tile([128, 16, S], f16, tag="st")
                sc = wpool.tile([128, 16, S], f16, tag="sc")

                tr = wpool.tile([128, 16, S], f16, tag="tr")
                nc.scalar.activation(tr[:, :, :], mhr[:, :, :], Act.Sigmoid)
                nc.vector.tensor_tensor(t2[:, :, :], tr[:, :, :],
                                        mhh[:, :, :], Alu.mult)
                nc.vector.tensor_tensor(t3[:, :, :], t2[:, :, :],
                                        xhc[:, :, :], Alu.add)
                nc.scalar.activation(tz[:, :, :], mhz[:, :, :], Act.Sigmoid)
                nc.vector.tensor_tensor(uh[:, :, :], tz[:, :, :],
                                        t_hs[:, :, k, :], Alu.mult)
                nc.vector.tensor_scalar(st[:, :, :], tz[:, :, :], -1.0, 1.0,
                                        Alu.mult, Alu.add)
                if k + 2 < NK:
                    emit_xh_copy(k + 2)
                nc.scalar.activation(cg[:, :, :], t3[:, :, :], Act.Tanh)
                nc.vector.tensor_tensor(sc[:, :, :], st[:, :, :],
                                        cg[:, :, :], Alu.mult)
                nc.vector.tensor_tensor(t_hs[:, 0:8, k + 1, :],
                                        uh[:, 0:8, :], sc[:, 0:8, :], Alu.add)
                nc.vector.tensor_tensor(t_hs[:, 8:16, k + 1, :],
                                        uh[:, 8:16, :], sc[:, 8:16, :],
                                        Alu.add)
                gate_prio.__exit__(None, None, None)

                if k >= 4:
                    emit_setup()
                if not SCAN_ONLY and NK - CL <= k < NK:
                    emit_hp(k - (NK - CL))

            while setup_i[0] < len(setup_ops):
                emit_setup()

            # ---- attention epilogue ----
            if not SCAN_ONLY:
                nc.scalar.activation(t_c1[:, :, :, :], hp[:, :, :, :],
                                     Act.Tanh)

                emit_fill(40)
                ps_pair = [spool.tile([128, 2, N], f32, tag="scps",
                                      name=f"scps_{p}") for p in range(2)]
                for p in range(2):
                    nc.tensor.matmul(ps_pair[p][:, :, :], t_ones[:, :],
                                     t_vq[:, 2 * p:2 * p + 2, :],
                                     start=True, stop=False,
                                     skip_group_check=True)
                # Raw Chebyshev even/odd chains (plain TT links, 2x mode):
                # T_m = (2 T_2) T_{m-2} - T_{m-4}; lambda lives in the G tiles
                tmt = {1: t_c1}
                c2t = wpool.tile([128, BL, CL, S], f16, tag="tchw", bufs=3)
                nc.vector.tensor_tensor(c2t[:, :, :, :], t_c1[:, :, :, :],
                                        t_c1[:, :, :, :], Alu.mult)
                t2c = wpool.tile([128, BL, CL, S], f16, tag="tcheb", bufs=6)
                nc.vector.tensor_scalar(t2c[:, :, :, :], c2t[:, :, :, :],
                                        2.0, -1.0, Alu.mult, Alu.add)
                tmt[2] = t2c
                t2d = wpool.tile([128, BL, CL, S], f16, tag="tchw", bufs=3)
                nc.vector.tensor_scalar(t2d[:, :, :, :], t2c[:, :, :, :],
                                        2.0, None, Alu.mult)

                def emit_cheb(m):  # T_m = t2d * T_{m-2} - T_{m-4}
                    tw = wpool.tile([128, BL, CL, S], f16, tag="tchw", bufs=3)
                    nc.vector.tensor_tensor(
                        tw[:, :, :, :], t2d[:, :, :, :],
                        tmt[m - 2][:, :, :, :], Alu.mult)
                    tn = wpool.tile([128, BL, CL, S], f16, tag="tcheb",
                                    bufs=6)
                    if m == 4:  # T_0 = 1
                        nc.vector.tensor_scalar(
                            tn[:, :, :, :], tw[:, :, :, :], 1.0, -1.0,
                            Alu.mult, Alu.add)
                    else:  # m=3 uses T_{-1} = T_1
                        nc.vector.tensor_tensor(
                            tn[:, :, :, :], tw[:, :, :, :],
                            tmt[m - 4 if m > 4 else 1][:, :, :, :],
                            Alu.subtract)
                    tmt[m] = tn

                for m in range(3, J + 1):
                    emit_cheb(m)
                for m in range(1, J + 1):
                    for b in range(BL):
                        nc.tensor.matmul(ps_pair[b // 2][:, b % 2, :],
                                         tmt[m][:, b, :, :], t_G[m - 1][:, b, :],
                                         start=False, stop=(m == J),
                                         skip_group_check=True)
                    emit_fill(4)

                # softmax over n (free dim) + final data multiply
                for b in range(BL):
                    ex = wpool.tile([128, N], f16, tag="expv")
                    nc.scalar.activation(ex[:, :], ps_pair[b // 2][:, b % 2, :],
                                         Act.Exp,
                                         accum_out=t_ssum[:, b:b + 1])
                    nc.vector.reciprocal(t_rinv[:, b:b + 1],
                                         t_ssum[:, b:b + 1])
                    alpha = wpool.tile([128, N], f16, tag="alphav")
                    nc.vector.tensor_scalar(
                        alpha[:, :], ex[:, :], t_rinv[:, b:b + 1], None,
                        Alu.mult)
                    dt_ = wpool.tile([128, N], f32, tag="dmul")
                    ot = wpool.tile([128, N], f32, tag="omul")
                    nc.sync.dma_start(out=dt_[:, :], in_=d_dataout.ap()[b, :, :])
                    eng = nc.vector if b % 2 == 0 else nc.gpsimd
                    eng.tensor_tensor(ot[:, :], dt_[:, :], alpha[:, :],
                                      Alu.mult)
                    nc.sync.dma_start(out=d_out.ap()[b, :, :], in_=ot[:, :])
                    if DEBUG:
                        nc.sync.dma_start(out=d_alp.ap()[b, :, :],
                                          in_=alpha[:, :])
            else:
                for b in range(BL):
                    dt_ = wpool.tile([128, N], f32, tag="dmul")
                    ot = wpool.tile([128, N], f32, tag="omul")
                    nc.sync.dma_start(out=dt_[:, :], in_=d_dataout.ap()[b, :, :])
                    nc.vector.tensor_tensor(ot[:, :], dt_[:, :], dt_[:, :],
                                            Alu.mult)
                    nc.sync.dma_start(out=d_out.ap()[b, :, :], in_=ot[:, :])

            if DEBUG:
                nc.sync.dma_start(out=d_hs.ap()[:, :, :, :],
                                  in_=t_hs[:, :, :, :])
                nc.sync.dma_start(out=d_sxd.ap()[:, :, :], in_=t_sx[:, :, :])
                if not SCAN_ONLY:
                    nc.sync.dma_start(out=d_qd.ap()[:, :, :], in_=t_q[:, :, :])
                    nc.sync.dma_start(out=d_cd.ap()[:, :, :, :],
                                      in_=t_c1[:, :, :, :])

    nc.compile()
    return nc


def _prep_inputs(data, h0, gru_kernel, gru_rkernel, gru_bias,
                 w1_w, w1_b, w2_w, w2_b, v_w, v_b):
    f16 = np.float16
    f32 = np.float32

    import ml_dtypes
    R_all = np.ascontiguousarray(
        gru_rkernel.reshape(4, 128, 12, 128).transpose(1, 0, 2, 3))
    R8_l = np.ascontiguousarray(
        R_all[:, :, 0:8, :].astype(ml_dtypes.float8_e4m3).reshape(
            128, 2, 2, 8, 128))
    Rh_l = R_all[:, :, 8:12, :].astype(f16)
    K_l = np.ascontiguousarray(
        gru_kernel.astype(f16).reshape(2, 128, 12, 128).transpose(1, 0, 2, 3))
    w1_l = w1_w.astype(f16)
    w2_l = np.ascontiguousarray(
        w2_w.astype(f16).reshape(4, 128, 128).transpose(1, 0, 2))
    ident = np.eye(128, dtype=f16)
    ones = np.ones((128, 128), f16)

    b_in, b_rec = gru_bias[0].astype(f32), gru_bias[1].astype(f32)
    bzr = (b_in + b_rec)[:1024].reshape(8, 128).T.copy()      # [128, 8]
    bh = b_in[1024:].reshape(4, 128).T.copy()                 # [128, 4]
    brech16 = np.zeros((128, 16), f16)
    for j in range(4):
        for bb in range(4):
            brech16[:, 4 * j + bb] = b_rec[1024 + 128 * j:1024 + 128 * (j + 1)]
    brech_rep = np.ascontiguousarray(np.broadcast_to(
        brech16[:, :, None, None], (128, 16, W // (T // S) + S, T // S))
        ).astype(f16)
    bu = (w1_b + w2_b).astype(f32).reshape(128, 1)
    v16 = v_w[:, 0].astype(f32).reshape(128, 1)

    data16 = data.astype(f16)

    # output row perm: psum partition p = S*o + i  <->  t' = CL*i + o
    pp = np.arange(128)
    tprime = (T // S) * (pp % S) + pp // S

    per_core = []
    for c in range(NC):
        sl = slice(BL * c, BL * (c + 1))
        h0t = np.zeros((128, 16, S), f16)
        for j in range(4):
            for bb in range(BL):
                for ch in range(W // (T // S)):
                    h0t[:, 4 * j + bb, ch] = \
                        h0[BL * c + bb, 128 * j:128 * (j + 1)]
        # dataout[l, p, :] = data[bg, tg] with flat = t'*32 + 4c + l
        dataout = np.empty((BL, 128, N), f32)
        for l in range(BL):
            flat = tprime * 32 + 4 * c + l
            dataout[l] = data[flat // 128, flat % 128, :]
        per_core.append({
            "data16": data16[sl], "dataout": dataout, "h0t": h0t,
            "R8_l": R8_l, "Rh_l": Rh_l, "K_l": K_l, "w1_l": w1_l,
            "w2_l": w2_l, "ident": ident, "ones": ones, "bias_zr": bzr,
            "bias_h": bh, "brech_rep": brech_rep, "bias_u": bu, "v16": v16,
        })
    return per_core


def kernel(**inputs):
    from concourse.bass_utils import run_bass_kernel_spmd

    if "nc" not in _CACHE:
        _CACHE["nc"] = _build()
    nc = _CACHE["nc"]

    args = {k: np.asarray(v) for k, v in inputs.items()}
    per_core = _prep_inputs(
        args["data"], args["h0"], args["gru_kernel"], args["gru_rkernel"],
        args["gru_bias"], args["w1_w"], args["w1_b"], args["w2_w"],
        args["w2_b"], args["v_w"], args["v_b"])

    if "warm" not in _CACHE:
        run_bass_kernel_spmd(nc, per_core, core_ids=list(range(NC)))
        _CACHE["warm"] = True
    res = run_bass_kernel_spmd(nc, per_core, core_ids=list(range(NC)))
    _CACHE["last_res"] = res

    pp = np.arange(128)
    tprime = (T // S) * (pp % S) + pp // S
    out = np.empty((B, T, N), np.float32)
    for c in range(NC):
        o = res.results[c]["out"]
        for l in range(BL):
            flat = tprime * 32 + 4 * c + l
            out[flat // 128, flat % 128, :] = o[l]
    return out


# revision 23
# speedup vs baseline: 1.0907x; 1.0445x over previous
"""Trainium2 Bass kernel for nn_Encoder (GRU + input attention), v2.

Shapes (hardcoded): B=32, T=128, N=256, H=512; 8 NeuronCores, batch
sharded 4 examples/core (BL=4).

Two structural changes vs the classic lagged-pipeline formulation:

1. Chunked scan: the GRU map is contracting (the state forgets its
   initial condition at ~0.73x/step), so the T=128 sequence is split
   into S=16 chunks of 8 steps run in LOCKSTEP as extra gate columns;
   chunks with enough history warm up from h=0 over the previous
   chunk's last W=16 steps, while chunks 0 and 1 are held at h0 by a
   z-freeze (z-gate input +60 => z=1 => h'=h exactly in f16) until
   their first real step, making them exact. Serial steps: 24 instead
   of 128. Step k, chunk i processes t = 8*i + k - 16 (m-frame pads
   cover t<0).

2. Matmul-ized attention: with q = tanh(sx/2) (half-angle of
   a = tanh(sx)) and c = tanh(hp):
     tanh(sx+hp) = q + (1-q^2) * sum_{m>=1} lam_m (-q)^(m-1) T_m(c)
   (Chebyshev expansion of the Moebius map (a+c)/(1+ac) in c; the
   coefficients are geometric in q = a/(1+sqrt(1-a^2)) <= 0.87, far
   better than the naive series in a*c whose ratio reaches 0.99).
   Truncated at J=12 with least-squares-fitted taper lam_m; each term
   of the u-contraction is a matmul (stationary T_m(c), moving
   G_m = lam_m v (1-q^2) (-q)^(m-1)), so the 16.7M-element e=tanh(...)
   tensor of the direct formulation never materializes. The lam taper
   rides in per-m scaled (-q) tiles so the G-stream (Pool engine,
   during the scan) and the Chebyshev links (DVE even/odd chains,
   epilogue) are plain tensor_tensor ops in fast DVE modes.

Other tricks: sigmoid+tanh live in one activation table set (exp only
at the final softmax => 2 table loads total); fp8e4m3 z/r recurrent
weights, f16 h-gate weights; gate biases and the per-t x-projections
are folded into identity-matmul PSUM seeds; filler matmuls into a
scratch PSUM bank keep the PE clock at max (the cost model drops to
1.2GHz after any idle gap); the score PSUM partition p = S*o + i is a
permutation of t' = 8*i + o that the host prep undoes, which also
absorbs the reference's alpha.reshape(-1, T, N) flat-reindex quirk.

Layouts per core:
  t_hs    [128, 16, 25, 16] f16  [p, 4j+b, slot, i]; slot k+1 = state
                                 after step k; hs(t) at slot t%8+17,
                                 chunk t//8.
  t_addmx [128, 48, 18, 8]  f16  [p, gate g, m, kr]; t_pad=8m+kr=t+16;
                                 g: z 0:16, r 16:32, h-seed 32:48
                                 (g=4*uc+b); m<2 is the warmup pad.
  t_xh    [128, 16, 18, 8]  f16  same t addressing.
  gates   [128, 16, 16] f32 PSUM (4uc+b, i), seeded from the addmx
                                 slice [:, g, ms:ms+16, kr] (ms=k//8).
  score   [128, 2, 256] f32 PSUM two b per bank, one seed matmul.
"""

import os
import sys

import numpy as np

for _p in ("/root/.axon_site", "/root/.axon_site/_ro/trn_rl_repo",
           "/root/.axon_site/_ro/pypackages", "/opt/trn_rl_repo",
           "/opt/pypackages"):
    if os.path.isdir(_p) and _p not in sys.path:
        sys.path.append(_p)

B, T, N, H = 32, 128, 256, 512
NC = 8           # cores
BL = B // NC     # batch per core (4)
S = 16           # scan chunks
CL = T // S      # chunk length (8)
W = 16           # warmup steps
NP = W // CL     # pad frames (2)
NM = NP + S      # m-frames in addmx (18)
NK = W + CL      # serial steps (24)

LAM = np.array([
    0.99985935, 0.99958375, 0.99593208, 0.99216078, 0.96761417,
    0.95277552, 0.86574698, 0.8334939, 0.63463465, 0.59383626,
    0.29231448, 0.26749125])
J = len(LAM)

_CACHE = {}
DEBUG = os.environ.get("NN_ENC_DEBUG", "0") == "1"
SCAN_ONLY = os.environ.get("NN_ENC_SCAN_ONLY", "0") == "1"


def _build():
    import concourse.bass as bass
    import concourse.bacc as bacc
    import concourse.tile as tile
    import concourse.mybir as mybir

    f16 = mybir.dt.float16
    f32 = mybir.dt.float32
    f8 = mybir.dt.float8e4
    Alu = mybir.AluOpType
    Act = mybir.ActivationFunctionType

    nc = bacc.Bacc("TRN2", target_bir_lowering=False, debug=False)

    # ---- dram I/O ----
    d_data16 = nc.dram_tensor("data16", [BL, T, N], f16, kind="ExternalInput")
    d_dataout = nc.dram_tensor("dataout", [BL, 128, N], f32, kind="ExternalInput")
    d_h0t = nc.dram_tensor("h0t", [128, 16, S], f16, kind="ExternalInput")
    d_R8 = nc.dram_tensor("R8_l", [128, 2, 2, 8, 128], f8, kind="ExternalInput")
    d_Rh = nc.dram_tensor("Rh_l", [128, 4, 4, 128], f16, kind="ExternalInput")
    d_K = nc.dram_tensor("K_l", [128, 2, 12, 128], f16, kind="ExternalInput")
    d_w1 = nc.dram_tensor("w1_l", [128, 128], f16, kind="ExternalInput")
    d_w2 = nc.dram_tensor("w2_l", [128, 4, 128], f16, kind="ExternalInput")
    d_ident = nc.dram_tensor("ident", [128, 128], f16, kind="ExternalInput")
    d_ones = nc.dram_tensor("ones", [128, 128], f16, kind="ExternalInput")
    d_bzr = nc.dram_tensor("bias_zr", [128, 8], f32, kind="ExternalInput")
    d_bh = nc.dram_tensor("bias_h", [128, 4], f32, kind="ExternalInput")
    d_brech = nc.dram_tensor("brech_rep", [128, 16, NM, CL], f16,
                             kind="ExternalInput")
    d_bu = nc.dram_tensor("bias_u", [128, 1], f32, kind="ExternalInput")
    d_v16 = nc.dram_tensor("v16", [128, 1], f32, kind="ExternalInput")
    d_out = nc.dram_tensor("out", [BL, 128, N], f32, kind="ExternalOutput")
    if DEBUG:
        d_hs = nc.dram_tensor("hs_dump", [128, 16, NK + 1, S], f16,
                              kind="ExternalOutput")
        d_sxd = nc.dram_tensor("sx_dump", [128, BL, N], f16,
                               kind="ExternalOutput")
        d_qd = nc.dram_tensor("q_dump", [128, BL, N], f16,
                              kind="ExternalOutput")
        d_cd = nc.dram_tensor("c_dump", [128, BL, CL, S], f16,
                              kind="ExternalOutput")
        d_alp = nc.dram_tensor("alpha_dump", [BL, 128, N], f16,
                               kind="ExternalOutput")

    with tile.TileContext(nc) as tc:
        with (
            tc.tile_pool(name="const", bufs=1) as cpool,
            tc.tile_pool(name="work", bufs=4) as wpool,
            tc.tile_pool(name="mh", bufs=1, space="PSUM") as mhpool,
            tc.tile_pool(name="mhz", bufs=1, space="PSUM") as mhzpool,
            tc.tile_pool(name="mhh", bufs=1, space="PSUM") as mhhpool,
            tc.tile_pool(name="bigps", bufs=2, space="PSUM") as bpool,
            tc.tile_pool(name="scps", bufs=2, space="PSUM") as spool,
            tc.tile_pool(name="fill", bufs=1, space="PSUM") as fpool,
        ):
            # ---- persistent tiles ----
            t_R8 = cpool.tile([128, 2, 2, 8, 128], f8)
            t_Rh = cpool.tile([128, 4, 4, 128], f16)
            t_K = cpool.tile([128, 2, 12, 128], f16)
            t_w1 = cpool.tile([128, 128], f16)
            t_w2 = cpool.tile([128, 4, 128], f16)
            t_ident = cpool.tile([128, 128], f16)
            t_ones = cpool.tile([128, 128], f16)
            t_bzr = cpool.tile([128, 8], f32)
            t_bh = cpool.tile([128, 4], f32)
            t_bu = cpool.tile([128, 1], f32)
            t_v = cpool.tile([128, 1], f32)
            t_d16 = [cpool.tile([128, N], f16, tag=f"d16_{b}", name=f"d16_{b}")
                     for b in range(BL)]
            t_dT = cpool.tile([128, 2, BL, 128], f16)   # dataT [p, nc2, b, t]
            t_addmx = cpool.tile([128, 48, NM, CL], f16)
            t_xh = cpool.tile([128, 16, NM, CL], f16)
            t_sx = cpool.tile([128, BL, N], f16)        # score_x' per b
            t_hs = cpool.tile([128, 16, NK + 1, S], f16)
            # attention setup
            t_q = cpool.tile([128, BL, N], f16)
            t_nq = cpool.tile([128, BL, N], f16)
            t_nqm = [cpool.tile([128, BL, N], f16, tag=f"nqm_{m}",
                                name=f"nqm_{m}") for m in range(2, J + 1)]
            t_q2 = cpool.tile([128, BL, N], f16)
            t_pf = cpool.tile([128, BL, N], f16)
            t_vq = cpool.tile([128, BL, N], f16)
            t_G = [cpool.tile([128, BL, N], f16, tag=f"G_{m}", name=f"G_{m}")
                   for m in range(1, J + 1)]
            t_c1 = cpool.tile([128, BL, CL, S], f16)
            t_ssum = cpool.tile([128, BL], f32)
            t_rinv = cpool.tile([128, BL], f32)

            # warm up the PE clock while DMAs land (fillers have no deps)
            t_fill0 = cpool.tile([128, 64], f16, name="fill_sb")
            nc.vector.memset(t_fill0[:, :], 1.0)

            # ---- DMA in (prologue deps first, epilogue-only last) ----
            nc.sync.dma_start(out=t_ident[:, :], in_=d_ident.ap()[:, :])
            for b in range(BL):
                nc.sync.dma_start(out=t_d16[b][:, :], in_=d_data16.ap()[b, :, :])
            nc.sync.dma_start(out=t_K[:, :, 0:4, :], in_=d_K.ap()[:, :, 0:4, :])
            nc.sync.dma_start(out=t_K[:, :, 4:8, :], in_=d_K.ap()[:, :, 4:8, :])
            nc.sync.dma_start(out=t_K[:, :, 8:12, :],
                              in_=d_K.ap()[:, :, 8:12, :])
            nc.sync.dma_start(out=t_w1[:, :], in_=d_w1.ap()[:, :])
            nc.sync.dma_start(out=t_bzr[:, :], in_=d_bzr.ap()[:, :])
            nc.sync.dma_start(out=t_bh[:, :], in_=d_bh.ap()[:, :])
            nc.sync.dma_start(out=t_bu[:, :], in_=d_bu.ap()[:, :])
            nc.sync.dma_start(out=t_R8[:, :, :, :, :], in_=d_R8.ap()[:, :, :, :, :])
            nc.sync.dma_start(out=t_Rh[:, :, :, :], in_=d_Rh.ap()[:, :, :, :])
            nc.sync.dma_start(out=t_addmx[:, 32:48, :, :],
                              in_=d_brech.ap()[:, :, :, :])
            nc.sync.dma_start(out=t_hs[:, :, 0, :], in_=d_h0t.ap()[:, :, :])
            nc.sync.dma_start(out=t_v[:, :], in_=d_v16.ap()[:, :])
            nc.sync.dma_start(out=t_w2[:, :, :], in_=d_w2.ap()[:, :, :])
            nc.sync.dma_start(out=t_ones[:, :], in_=d_ones.ap()[:, :])

            # warmup pads: z-freeze (sigmoid(60)=1 -> h'=h), r=0, xh=0
            nc.vector.memset(t_addmx[:, 0:16, 0:NP, :], 60.0)
            nc.vector.memset(t_addmx[:, 16:32, 0:NP, :], 0.0)
            nc.vector.memset(t_xh[:, :, 0:NP, :], 0.0)

            # ---- PE p-state fillers: harmless matmuls into a scratch
            # bank keep the tensor engine from dropping out of max clock
            # during dependency gaps (max clock needs 3us continuous) ----
            t_fill = fpool.tile([128, 64], f32, name="fill_ps")

            def emit_fill(n):
                for _ in range(n):
                    nc.tensor.matmul(t_fill[0:64, :], t_fill0[:, :],
                                     t_fill0[:, :], start=True, stop=False,
                                     skip_group_check=True)

            emit_fill(70)

            # ---- prologue: data^T  [p, nc2, b, t] ----
            for b in range(BL):
                for n2 in range(2):
                    ps = bpool.tile([128, 128], f16, tag="bigps")
                    nc.tensor.transpose(ps[:, :],
                                        t_d16[b][:, 128 * n2:128 * (n2 + 1)],
                                        t_ident[:, :])
                    nc.vector.tensor_copy(t_dT[:, n2, b, :], ps[:, :])

            # ---- prologue: mx = data @ K (+biases) -> addmx/xh ----
            for uc in range(12):
                ps = bpool.tile([128, BL, 128], f32, tag="bigps")
                for n2 in range(2):
                    nc.tensor.matmul(ps[:, :, :], t_K[:, n2, uc, :],
                                     t_dT[:, n2, :, :],
                                     start=(n2 == 0), stop=(n2 == 1))
                g, j = divmod(uc, 4)
                if g < 2:
                    dst, bias = t_addmx[:, 4 * uc:4 * uc + 4, NP:NM, :], \
                        t_bzr[:, uc:uc + 1]
                else:
                    dst, bias = t_xh[:, 4 * j:4 * j + 4, NP:NM, :], \
                        t_bh[:, j:j + 1]
                if uc % 2 == 0:
                    nc.scalar.activation(dst, ps[:, :, :], Act.Identity,
                                         bias=bias)
                else:
                    nc.vector.tensor_scalar(dst, ps[:, :, :], bias, None,
                                            Alu.add)

            # ---- prologue: score_x' ----
            for b in range(BL):
                ps = bpool.tile([128, N], f32, tag="bigps")
                nc.tensor.matmul(ps[:, :], t_w1[:, :], t_d16[b][:, :],
                                 start=True, stop=True)
                nc.scalar.activation(t_sx[:, b, :], ps[:, :],
                                     Act.Identity, bias=t_bu[:, :])

            # ---- attention setup ops (independent of the scan; emitted
            # interleaved into scan steps below to fill DVE idle time) ----
            setup_ops = []
            if not SCAN_ONLY:
                setup_ops = [
                    # q = tanh(sx/2) (half-angle of a=tanh(sx)); pref = 1-q^2
                    lambda: nc.scalar.activation(t_q[:, :, :], t_sx[:, :, :],
                                                 Act.Tanh, scale=0.5),
                    lambda: nc.gpsimd.tensor_tensor(
                        t_q2[:, :, :], t_q[:, :, :], t_q[:, :, :], Alu.mult),
                    lambda: nc.vector.tensor_scalar(
                        t_pf[:, :, :], t_q2[:, :, :], -1.0, 1.0,
                        Alu.mult, Alu.add),
                    lambda: nc.vector.tensor_scalar(
                        t_nq[:, :, :], t_q[:, :, :], -1.0, None, Alu.mult),
                    lambda: nc.vector.tensor_scalar(
                        t_vq[:, :, :], t_q[:, :, :], t_v[:, 0:1], None,
                        Alu.mult),
                    lambda: nc.vector.tensor_scalar(
                        t_G[0][:, :, :], t_pf[:, :, :], t_v[:, 0:1],
                        float(LAM[0]), Alu.mult, Alu.mult),
                ]
                # per-m scaled -q tiles carry the lambda taper so both the
                # Pool G-stream and the epilogue Chebyshev links are plain TT
                for m in range(2, J + 1):
                    def _s(m=m):
                        nc.vector.tensor_scalar(
                            t_nqm[m - 2][:, :, :], t_nq[:, :, :],
                            float(LAM[m - 1] / LAM[m - 2]), None, Alu.mult)
                    setup_ops.append(_s)
                for m in range(2, J + 1):
                    def _g(m=m):
                        nc.gpsimd.tensor_tensor(
                            t_G[m - 1][:, :, :], t_G[m - 2][:, :, :],
                            t_nqm[m - 2][:, :, :], Alu.mult)
                    setup_ops.append(_g)
            setup_i = [0]

            def emit_setup():
                if setup_i[0] < len(setup_ops):
                    setup_ops[setup_i[0]]()
                    setup_i[0] += 1

            # ---- main scan loop (NK serial steps) ----
            xhc_t = {}

            def emit_xh_copy(k):
                ms, kr = divmod(k, CL)
                xc = wpool.tile([128, 16, S], f16, tag="xhc", bufs=4)
                nc.vector.tensor_copy(xc[:, :, :], t_xh[:, :, ms:ms + S, kr])
                xhc_t[k] = xc

            emit_xh_copy(0)
            emit_xh_copy(1)
            uh_t, sc_t = {}, {}
            hp = bpool.tile([128, BL, CL, S], f32, tag="bigps", name="hp_ps")

            def emit_hp(o):
                # hp[u, b, o, i] = sum_h hs[t=8i+o, b, h] w2[h, u]
                for hc in range(4):
                    nc.tensor.matmul(hp[:, :, o, :], t_w2[:, hc, :],
                                     t_hs[:, 4 * hc:4 * hc + 4, o + 17, :],
                                     start=(hc == 0), stop=(hc == 3),
                                     skip_group_check=True)

            for k in range(NK):
                ms, kr = divmod(k, CL)

                mhr = mhpool.tile([128, 16, S], f32)
                mhz = mhzpool.tile([128, 16, S], f32)
                mhh = mhhpool.tile([128, 16, S], f32)
                # r bank first (chain-critical), then h, then z
                nc.tensor.matmul(mhr[:, :, :], t_ident[:, :],
                                 t_addmx[:, 16:32, ms:ms + S, kr],
                                 start=True, stop=False, skip_group_check=True)
                if k == 0:
                    for kc in range(4):
                        for uc in range(4):
                            nc.tensor.matmul(
                                mhr[:, 4 * uc:4 * uc + 4, :],
                                t_R8[:, kc // 2, kc % 2, uc + 4, :],
                                t_hs[:, 4 * kc:4 * kc + 4, 0, :],
                                start=False, stop=(uc == 3 and kc == 3),
                                skip_group_check=True)
                else:
                    # R^T h = R^T uh + R^T sc: the uh part runs on idle PE
                    # during the previous step's cg window; only the sc part
                    # waits, so hnew leaves the critical path
                    puh, psc = uh_t.pop(k - 1), sc_t.pop(k - 1)
                    for part in (puh, psc):
                        for kc in range(4):
                            for uc in range(4):
                                nc.tensor.matmul(
                                    mhr[:, 4 * uc:4 * uc + 4, :],
                                    t_R8[:, kc // 2, kc % 2, uc + 4, :],
                                    part[:, 4 * kc:4 * kc + 4, :],
                                    start=False,
                                    stop=(part is psc and uc == 3
                                          and kc == 3),
                                    skip_group_check=True)
                nc.tensor.matmul(mhh[:, :, :], t_ident[:, :],
                                 t_addmx[:, 32:48, ms:ms + S, kr],
                                 start=True, stop=False, skip_group_check=True)
                for uc in range(4):
                    for kc in range(4):
                        nc.tensor.matmul(mhh[:, 4 * uc:4 * uc + 4, :],
                                         t_Rh[:, kc, uc, :],
                                         t_hs[:, 4 * kc:4 * kc + 4, k, :],
                                         start=False,
                                         stop=(uc == 3 and kc == 3),
                                         skip_group_check=True)
                nc.tensor.matmul(mhz[:, :, :], t_ident[:, :],
                                 t_addmx[:, 0:16, ms:ms + S, kr],
                                 start=True, stop=False, skip_group_check=True)
                for uc in range(4):
                    for kc in range(4):
                        nc.tensor.matmul(mhz[:, 4 * uc:4 * uc + 4, :],
                                         t_R8[:, kc // 2, kc % 2, uc, :],
                                         t_hs[:, 4 * kc:4 * kc + 4, k, :],
                                         start=False,
                                         stop=(uc == 3 and kc == 3),
                                         skip_group_check=True)

                emit_fill(12)
                xhc = xhc_t.pop(k)

                gate_prio = tc.high_priority(offset=200000)
                gate_prio.__enter__()

                t2 = wpool.tile([128, 16, S], f16, tag="t2")
                t3 = wpool.tile([128, 16, S], f16, tag="t3")
                cg = wpool.tile([128, 16, S], f16, tag="cg")
                tz = wpool.tile([128, 16, S], f16, tag="tz")
                uh = wpool.tile([128, 16, S], f16, tag="uh")
                st = wpool.tile([128, 16, S], f16, tag="st")
                sc = wpool.tile([128, 16, S], f16, tag="sc")

                tr = wpool.tile([128, 16, S], f16, tag="tr")
                nc.scalar.activation(tr[:, :, :], mhr[:, :, :], Act.Sigmoid)
                nc.vector.tensor_tensor(t2[:, :, :], tr[:, :, :],
                                        mhh[:, :, :], Alu.mult)
                nc.vector.tensor_tensor(t3[:, :, :], t2[:, :, :],
                                        xhc[:, :, :], Alu.add)
                nc.scalar.activation(tz[:, :, :], mhz[:, :, :], Act.Sigmoid)
                nc.vector.tensor_tensor(uh[:, :, :], tz[:, :, :],
                                        t_hs[:, :, k, :], Alu.mult)
                nc.vector.tensor_scalar(st[:, :, :], tz[:, :, :], -1.0, 1.0,
                                        Alu.mult, Alu.add)
                if k + 2 < NK:
                    emit_xh_copy(k + 2)
                nc.scalar.activation(cg[:, :, :], t3[:, :, :], Act.Tanh)
                nc.vector.tensor_tensor(sc[:, :, :], st[:, :, :],
                                        cg[:, :, :], Alu.mult)
                nc.vector.tensor_tensor(t_hs[:, 0:8, k + 1, :],
                                        uh[:, 0:8, :], sc[:, 0:8, :], Alu.add)
                nc.vector.tensor_tensor(t_hs[:, 8:16, k + 1, :],
                                        uh[:, 8:16, :], sc[:, 8:16, :],
                                        Alu.add)
                uh_t[k], sc_t[k] = uh, sc
                gate_prio.__exit__(None, None, None)

                if k >= 4:
                    emit_setup()
                if not SCAN_ONLY and NK - CL <= k < NK:
                    emit_hp(k - (NK - CL))

            while setup_i[0] < len(setup_ops):
                emit_setup()

            # ---- attention epilogue ----
            if not SCAN_ONLY:
                nc.scalar.activation(t_c1[:, :, :, :], hp[:, :, :, :],
                                     Act.Tanh)

                emit_fill(40)
                ps_pair = [spool.tile([128, 2, N], f32, tag="scps",
                                      name=f"scps_{p}") for p in range(2)]
                for p in range(2):
                    nc.tensor.matmul(ps_pair[p][:, :, :], t_ones[:, :],
                                     t_vq[:, 2 * p:2 * p + 2, :],
                                     start=True, stop=False,
                                     skip_group_check=True)
                # Raw Chebyshev even/odd chains (plain TT links, 2x mode):
                # T_m = (2 T_2) T_{m-2} - T_{m-4}; lambda lives in the G tiles
                tmt = {1: t_c1}
                c2t = wpool.tile([128, BL, CL, S], f16, tag="tchw", bufs=3)
                nc.vector.tensor_tensor(c2t[:, :, :, :], t_c1[:, :, :, :],
                                        t_c1[:, :, :, :], Alu.mult)
                t2c = wpool.tile([128, BL, CL, S], f16, tag="tcheb", bufs=6)
                nc.vector.tensor_scalar(t2c[:, :, :, :], c2t[:, :, :, :],
                                        2.0, -1.0, Alu.mult, Alu.add)
                tmt[2] = t2c
                t2d = wpool.tile([128, BL, CL, S], f16, tag="tchw", bufs=3)
                nc.vector.tensor_scalar(t2d[:, :, :, :], t2c[:, :, :, :],
                                        2.0, None, Alu.mult)

                def emit_cheb(m):  # T_m = t2d * T_{m-2} - T_{m-4}
                    tw = wpool.tile([128, BL, CL, S], f16, tag="tchw", bufs=3)
                    nc.vector.tensor_tensor(
                        tw[:, :, :, :], t2d[:, :, :, :],
                        tmt[m - 2][:, :, :, :], Alu.mult)
                    tn = wpool.tile([128, BL, CL, S], f16, tag="tcheb",
                                    bufs=6)
                    if m == 4:  # T_0 = 1
                        nc.vector.tensor_scalar(
                            tn[:, :, :, :], tw[:, :, :, :], 1.0, -1.0,
                            Alu.mult, Alu.add)
                    else:  # m=3 uses T_{-1} = T_1
                        nc.vector.tensor_tensor(
                            tn[:, :, :, :], tw[:, :, :, :],
                            tmt[m - 4 if m > 4 else 1][:, :, :, :],
                            Alu.subtract)
                    tmt[m] = tn

                for m in range(3, J + 1):
                    emit_cheb(m)
                for m in range(1, J + 1):
                    for b in range(BL):
                        nc.tensor.matmul(ps_pair[b // 2][:, b % 2, :],
                                         tmt[m][:, b, :, :], t_G[m - 1][:, b, :],
                                         start=False, stop=(m == J),
                                         skip_group_check=True)
                    emit_fill(4)

                # softmax over n (free dim) + final data multiply
                for b in range(BL):
                    ex = wpool.tile([128, N], f16, tag="expv")
                    nc.scalar.activation(ex[:, :], ps_pair[b // 2][:, b % 2, :],
                                         Act.Exp,
                                         accum_out=t_ssum[:, b:b + 1])
                    nc.vector.reciprocal(t_rinv[:, b:b + 1],
                                         t_ssum[:, b:b + 1])
                    alpha = wpool.tile([128, N], f16, tag="alphav")
                    nc.vector.tensor_scalar(
                        alpha[:, :], ex[:, :], t_rinv[:, b:b + 1], None,
                        Alu.mult)
                    dt_ = wpool.tile([128, N], f32, tag="dmul")
                    ot = wpool.tile([128, N], f32, tag="omul")
                    nc.sync.dma_start(out=dt_[:, :], in_=d_dataout.ap()[b, :, :])
                    eng = nc.vector if b % 2 == 0 else nc.gpsimd
                    eng.tensor_tensor(ot[:, :], dt_[:, :], alpha[:, :],
                                      Alu.mult)
                    nc.sync.dma_start(out=d_out.ap()[b, :, :], in_=ot[:, :])
                    if DEBUG:
                        nc.sync.dma_start(out=d_alp.ap()[b, :, :],
                                          in_=alpha[:, :])
            else:
                for b in range(BL):
                    dt_ = wpool.tile([128, N], f32, tag="dmul")
                    ot = wpool.tile([128, N], f32, tag="omul")
                    nc.sync.dma_start(out=dt_[:, :], in_=d_dataout.ap()[b, :, :])
                    nc.vector.tensor_tensor(ot[:, :], dt_[:, :], dt_[:, :],
                                            Alu.mult)
                    nc.sync.dma_start(out=d_out.ap()[b, :, :], in_=ot[:, :])

            if DEBUG:
                nc.sync.dma_start(out=d_hs.ap()[:, :, :, :],
                                  in_=t_hs[:, :, :, :])
                nc.sync.dma_start(out=d_sxd.ap()[:, :, :], in_=t_sx[:, :, :])
                if not SCAN_ONLY:
                    nc.sync.dma_start(out=d_qd.ap()[:, :, :], in_=t_q[:, :, :])
                    nc.sync.dma_start(out=d_cd.ap()[:, :, :, :],
                                      in_=t_c1[:, :, :, :])

    nc.compile()
    return nc


def _prep_inputs(data, h0, gru_kernel, gru_rkernel, gru_bias,
                 w1_w, w1_b, w2_w, w2_b, v_w, v_b):
    f16 = np.float16
    f32 = np.float32

    import ml_dtypes
    R_all = np.ascontiguousarray(
        gru_rkernel.reshape(4, 128, 12, 128).transpose(1, 0, 2, 3))
    R8_l = np.ascontiguousarray(
        R_all[:, :, 0:8, :].astype(ml_dtypes.float8_e4m3).reshape(
            128, 2, 2, 8, 128))
    Rh_l = R_all[:, :, 8:12, :].astype(f16)
    K_l = np.ascontiguousarray(
        gru_kernel.astype(f16).reshape(2, 128, 12, 128).transpose(1, 0, 2, 3))
    w1_l = w1_w.astype(f16)
    w2_l = np.ascontiguousarray(
        w2_w.astype(f16).reshape(4, 128, 128).transpose(1, 0, 2))
    ident = np.eye(128, dtype=f16)
    ones = np.ones((128, 128), f16)

    b_in, b_rec = gru_bias[0].astype(f32), gru_bias[1].astype(f32)
    bzr = (b_in + b_rec)[:1024].reshape(8, 128).T.copy()      # [128, 8]
    bh = b_in[1024:].reshape(4, 128).T.copy()                 # [128, 4]
    brech16 = np.zeros((128, 16), f16)
    for j in range(4):
        for bb in range(4):
            brech16[:, 4 * j + bb] = b_rec[1024 + 128 * j:1024 + 128 * (j + 1)]
    brech_rep = np.ascontiguousarray(np.broadcast_to(
        brech16[:, :, None, None], (128, 16, W // (T // S) + S, T // S))
        ).astype(f16)
    bu = (w1_b + w2_b).astype(f32).reshape(128, 1)
    v16 = v_w[:, 0].astype(f32).reshape(128, 1)

    data16 = data.astype(f16)

    # output row perm: psum partition p = S*o + i  <->  t' = CL*i + o
    pp = np.arange(128)
    tprime = (T // S) * (pp % S) + pp // S

    per_core = []
    for c in range(NC):
        sl = slice(BL * c, BL * (c + 1))
        h0t = np.zeros((128, 16, S), f16)
        for j in range(4):
            for bb in range(BL):
                for ch in range(W // (T // S)):
                    h0t[:, 4 * j + bb, ch] = \
                        h0[BL * c + bb, 128 * j:128 * (j + 1)]
        # dataout[l, p, :] = data[bg, tg] with flat = t'*32 + 4c + l
        dataout = np.empty((BL, 128, N), f32)
        for l in range(BL):
            flat = tprime * 32 + 4 * c + l
            dataout[l] = data[flat // 128, flat % 128, :]
        per_core.append({
            "data16": data16[sl], "dataout": dataout, "h0t": h0t,
            "R8_l": R8_l, "Rh_l": Rh_l, "K_l": K_l, "w1_l": w1_l,
            "w2_l": w2_l, "ident": ident, "ones": ones, "bias_zr": bzr,
            "bias_h": bh, "brech_rep": brech_rep, "bias_u": bu, "v16": v16,
        })
    return per_core


def kernel(**inputs):
    from concourse.bass_utils import run_bass_kernel_spmd

    if "nc" not in _CACHE:
        _CACHE["nc"] = _build()
    nc = _CACHE["nc"]

    args = {k: np.asarray(v) for k, v in inputs.items()}
    per_core = _prep_inputs(
        args["data"], args["h0"], args["gru_kernel"], args["gru_rkernel"],
        args["gru_bias"], args["w1_w"], args["w1_b"], args["w2_w"],
        args["w2_b"], args["v_w"], args["v_b"])

    if "warm" not in _CACHE:
        run_bass_kernel_spmd(nc, per_core, core_ids=list(range(NC)))
        _CACHE["warm"] = True
    res = run_bass_kernel_spmd(nc, per_core, core_ids=list(range(NC)))
    _CACHE["last_res"] = res

    pp = np.arange(128)
    tprime = (T // S) * (pp % S) + pp // S
    out = np.empty((B, T, N), np.float32)
    for c in range(NC):
        o = res.results[c]["out"]
        for l in range(BL):
            flat = tprime * 32 + 4 * c + l
            out[flat // 128, flat % 128, :] = o[l]
    return out
